# revision 1
# baseline (speedup 1.0000x reference)
"""AttentionBlock (GroupNorm + 1x1-conv QKV self-attention + residual) on 8 TRN2 cores.

Data-parallel over batch: 16 samples -> 2 per NeuronCore, no collectives.
Per-sample layout is [C, S] = [512, 1024] with channels on partitions (4 chunks
of 128). Scores are computed transposed (St[t,s] = K^T Q) so every matmul reads
its operands in natural layout; softmax row-sums over the partition axis are
done with a ones-vector matmul on the PE, and the 1/rowsum normalization is
broadcast across partitions with a K=1 matmul. Weight transposes and the
additive biases bv/bo are folded on the host (xpb = x + Wo@bv + bo).
Big matmuls run in bf16 (full PE rate) with fp32 PSUM accumulation; the
GroupNorm statistics, softmax normalization, and residual path stay fp32.
"""

import numpy as np

N, C, H, W = 16, 512, 32, 32
S = H * W                      # 1024
NCORES = 8
NSAMP = N // NCORES            # 2 samples per core
NCCH = C // 128                # 4 channel chunks
NSH = S // 512                 # 2 free-dim halves
NT = S // 128                  # 8 key tiles
GROUPS = 32
GSIZE = (C // GROUPS) * S      # 16384 elements per group
EPS = 1e-5
SCALE = float(C) ** -0.5

_CACHE = {}


def _build():
    import concourse.bass as bass
    import concourse.tile as tile
    from concourse import bacc, mybir
    from contextlib import ExitStack

    F32 = mybir.dt.float32
    BF16 = mybir.dt.bfloat16
    AF = mybir.ActivationFunctionType
    OP = mybir.AluOpType

    nc = bacc.Bacc("TRN2", target_bir_lowering=False, debug=False,
                   num_devices=NCORES)

    x_ext = nc.declare_dram_parameter("x", [NSAMP, C, S], mybir.dt.bfloat16, isOutput=False)
    xpb_ext = nc.declare_dram_parameter("xpb", [NSAMP, C, S], F32, isOutput=False)
    wqT_ext = nc.declare_dram_parameter("wqT", [C, C], mybir.dt.bfloat16, isOutput=False)
    wkT_ext = nc.declare_dram_parameter("wkT", [C, C], mybir.dt.bfloat16, isOutput=False)
    wvT_ext = nc.declare_dram_parameter("wvT", [C, C], mybir.dt.bfloat16, isOutput=False)
    woT_ext = nc.declare_dram_parameter("woT", [C, C], mybir.dt.bfloat16, isOutput=False)
    bqt_ext = nc.declare_dram_parameter("bqt", [128, NCCH], F32, isOutput=False)
    bkt_ext = nc.declare_dram_parameter("bkt", [128, NCCH], F32, isOutput=False)
    gt_ext = nc.declare_dram_parameter("gt", [128, NCCH], F32, isOutput=False)
    bt_ext = nc.declare_dram_parameter("bt", [128, NCCH], F32, isOutput=False)
    gm8_ext = nc.declare_dram_parameter("gm8", [128, 8], F32, isOutput=False)
    gm8T_ext = nc.declare_dram_parameter("gm8T", [8, 128], F32, isOutput=False)
    out_ext = nc.declare_dram_parameter("out", [NSAMP, C, S], F32, isOutput=True)

    def mm(psum, lhsT, rhs, start, stop):
        nc.tensor.matmul(psum, lhsT, rhs, start=start, stop=stop)

    with ExitStack() as ctx:
        tc = ctx.enter_context(tile.TileContext(nc))

        singles = ctx.enter_context(tc.tile_pool(name="singles", bufs=1))
        xh_pool = ctx.enter_context(tc.tile_pool(name="xh", bufs=4))
        h_pool = ctx.enter_context(tc.tile_pool(name="h", bufs=8))
        q_pool = ctx.enter_context(tc.tile_pool(name="q", bufs=4))
        k_pool = ctx.enter_context(tc.tile_pool(name="k", bufs=4))
        vt_pool = ctx.enter_context(tc.tile_pool(name="vt", bufs=8))
        e_pool = ctx.enter_context(tc.tile_pool(name="e", bufs=8))
        hn_pool = ctx.enter_context(tc.tile_pool(name="hn", bufs=4))
        xo_pool = ctx.enter_context(tc.tile_pool(name="xo", bufs=2))
        rbc_pool = ctx.enter_context(tc.tile_pool(name="rbc", bufs=2))
        small = ctx.enter_context(tc.tile_pool(name="small", bufs=4))
        dram = ctx.enter_context(tc.tile_pool(name="dram", bufs=2,
                                              space="DRAM"))
        pmm = ctx.enter_context(tc.tile_pool(name="pmm", bufs=6, space="PSUM"))
        prs = ctx.enter_context(tc.tile_pool(name="prs", bufs=2, space="PSUM"))

        # --- prefetch x for both samples first: DMA head start so GroupNorm
        # --- statistics can begin before the (larger) weight DMAs finish
        # DMA priority order: sample-0 x (gates GroupNorm), small constants,
        # then wq/wk (gate the first conv), sample-1 x, then wv/wo.
        xcs = [[None] * NCCH for _ in range(NSAMP)]
        def fetch_x(n):
            for ci in range(NCCH):
                xc = xh_pool.tile([128, S], BF16, tag="xh", name="xh")
                nc.sync.dma_start(out=xc,
                                  in_=x_ext[n, ci * 128:(ci + 1) * 128, :])
                xcs[n][ci] = xc
        fetch_x(0)
        bqt_sb = singles.tile([128, NCCH], F32, tag="bqt", name="bqt")
        nc.sync.dma_start(out=bqt_sb, in_=bqt_ext[:])
        bkt_sb = singles.tile([128, NCCH], F32, tag="bkt", name="bkt")
        nc.sync.dma_start(out=bkt_sb, in_=bkt_ext[:])
        gt_sb = singles.tile([128, NCCH], F32, tag="gt", name="gt")
        nc.sync.dma_start(out=gt_sb, in_=gt_ext[:])
        bt_sb = singles.tile([128, NCCH], F32, tag="bt", name="bt")
        nc.sync.dma_start(out=bt_sb, in_=bt_ext[:])
        gm8_sb = singles.tile([128, 8], F32, tag="gm8", name="gm8")
        nc.sync.dma_start(out=gm8_sb, in_=gm8_ext[:])
        gm8T_sb = singles.tile([8, 128], F32, tag="gm8T", name="gm8T")
        nc.sync.dma_start(out=gm8T_sb, in_=gm8T_ext[:])
        ones_k = singles.tile([128, 1], BF16, tag="ones_k", name="ones_k")
        nc.vector.memset(ones_k, 1.0)
        eps_sb = singles.tile([128, 1], F32, tag="eps", name="eps")
        nc.vector.memset(eps_sb, EPS)
        w_sb = {}
        def fetch_w(name, ext):
            t = singles.tile([128, NCCH, C], BF16, tag=name, name=name)
            nc.sync.dma_start(out=t, in_=ext.ap().rearrange(
                "(a p) o -> p a o", p=128))
            w_sb[name] = t
        fetch_w("wqT", wqT_ext)
        fetch_w("wkT", wkT_ext)
        fetch_x(1)
        fetch_w("wvT", wvT_ext)
        fetch_w("woT", woT_ext)

        def gn_stats(n):
            """DVE-side GroupNorm statistics: per-partition [sum, sum_sq]
            for all 4 chunks packed into one [128, 8] tile."""
            ss8 = small.tile([128, 2 * NCCH], F32, tag="ss8", name="ss8")
            for ci in range(NCCH):
                xc = xcs[n][ci]
                st6 = small.tile([128, 2, nc.vector.BN_STATS_DIM], F32,
                                 tag="st6", name="st6")
                xcr = xc.rearrange("p (a b) -> p a b", b=512)
                for a in range(2):
                    nc.vector.bn_stats(out=st6[:, a, :], in_=xcr[:, a, :])
                mv = small.tile([128, 2], F32, tag="mv", name="mv")
                nc.vector.bn_aggr(out=mv, in_=st6)
                m2p = small.tile([128, 1], F32, tag="m2p", name="m2p")
                nc.vector.tensor_mul(m2p, mv[:, 0:1], mv[:, 0:1])
                nc.vector.tensor_add(m2p, m2p, mv[:, 1:2])
                nc.scalar.mul(ss8[:, 2 * ci:2 * ci + 1], mv[:, 0:1], float(S))
                nc.scalar.mul(ss8[:, 2 * ci + 1:2 * ci + 2], m2p, float(S))
            return ss8

        def gn_finish(n, ss8):
            """Group reduce/broadcast via one matmul pair, then the affine
            H = x*a + b with the per-chunk chain vectorized to [128,4]."""
            gp8 = pmm.tile([8, 2 * NCCH], F32, tag="m", name="m")
            nc.tensor.matmul(gp8, gm8_sb, ss8, start=True, stop=True)
            gs8 = small.tile([8, 2 * NCCH], F32, tag="gs8", name="gs8")
            nc.vector.tensor_copy(gs8, gp8)
            pp8 = pmm.tile([128, 2 * NCCH], F32, tag="m", name="m")
            nc.tensor.matmul(pp8, gm8T_sb, gs8, start=True, stop=True)
            meanex8 = small.tile([128, 2 * NCCH], F32, tag="meanex8",
                                 name="meanex8")
            nc.scalar.mul(meanex8, pp8, 1.0 / GSIZE)
            mev = meanex8.rearrange("p (c two) -> p two c", two=2)
            mean4, ex24 = mev[:, 0, :], mev[:, 1, :]
            var4 = small.tile([128, NCCH], F32, tag="var4", name="var4")
            nc.vector.tensor_mul(var4, mean4, mean4)
            nc.vector.tensor_sub(var4, ex24, var4)
            sd4 = small.tile([128, NCCH], F32, tag="sd4", name="sd4")
            nc.scalar.activation(sd4, var4, AF.Sqrt, bias=eps_sb)
            rstd4 = small.tile([128, NCCH], F32, tag="rstd4", name="rstd4")
            nc.vector.reciprocal_approx_fast(rstd4, sd4)
            ga4 = small.tile([128, NCCH], F32, tag="ga4", name="ga4")
            nc.vector.tensor_mul(ga4, gt_sb, rstd4)
            gb4 = small.tile([128, NCCH], F32, tag="gb4", name="gb4")
            nc.vector.tensor_mul(gb4, mean4, ga4)
            nc.vector.tensor_sub(gb4, bt_sb, gb4)
            h_sb = []
            for ci in range(NCCH):
                hb = h_pool.tile([128, S], BF16, tag="h", name="h")
                nc.vector.tensor_scalar(out=hb, in0=xcs[n][ci],
                                        scalar1=ga4[:, ci:ci + 1],
                                        scalar2=gb4[:, ci:ci + 1],
                                        op0=OP.mult, op1=OP.add)
                h_sb.append(hb)
            return h_sb

        def emit_qkv(n, h_sb):
            q_sb, k_sb = [], []
            for wname, bias_sb, dst in (("wqT", bqt_sb, q_sb),
                                        ("wkT", bkt_sb, k_sb)):
                for oi in range(NCCH):
                    qt = (q_pool if wname == "wqT" else k_pool).tile(
                        [128, S], BF16, tag="qk",
                        name="q" if wname == "wqT" else "k")
                    for sh in range(NSH):
                        ps = pmm.tile([128, 512], F32, tag="m", name="m")
                        for ci in range(NCCH):
                            mm(ps, w_sb[wname][:, ci, oi * 128:(oi + 1) * 128],
                               h_sb[ci][:, sh * 512:(sh + 1) * 512],
                               start=ci == 0, stop=ci == NCCH - 1)
                        nc.scalar.activation(qt[:, sh * 512:(sh + 1) * 512],
                                             ps, AF.Identity,
                                             bias=bias_sb[:, oi:oi + 1])
                    dst.append(qt)
            vt_sb = []
            for ti in range(NT):
                vt = vt_pool.tile([128, C], BF16, tag="vt", name="vt")
                ps = pmm.tile([128, 512], F32, tag="m", name="m")
                for ci in range(NCCH):
                    mm(ps, h_sb[ci][:, ti * 128:(ti + 1) * 128],
                       w_sb["wvT"][:, ci, :],
                       start=ci == 0, stop=ci == NCCH - 1)
                nc.vector.tensor_copy(vt, ps)
                vt_sb.append(vt)
            return q_sb, k_sb, vt_sb

        def emit_scores(n, q_sb, k_sb):
            """St[t,s] = K^T Q, E = exp(scale*St); rowsum over t via a
            ones-column matmul, delayed one tile so the PE never waits on
            the Exp activation."""
            e_sb = []
            rs_ps = [prs.tile([1, 512], F32, tag="r", name="r")
                     for _ in range(NSH)]
            pend = []
            for ti in range(NT):
                et = e_pool.tile([128, S], BF16, tag="e", name="e")
                for sh in range(NSH):
                    ps = pmm.tile([128, 512], F32, tag="m", name="m")
                    for ci in range(NCCH):
                        mm(ps, k_sb[ci][:, ti * 128:(ti + 1) * 128],
                           q_sb[ci][:, sh * 512:(sh + 1) * 512],
                           start=ci == 0, stop=ci == NCCH - 1)
                    nc.scalar.activation(et[:, sh * 512:(sh + 1) * 512], ps,
                                         AF.Exp, scale=SCALE)
                for prev_ti, prev_et in pend:
                    for sh in range(NSH):
                        mm(rs_ps[sh], ones_k,
                           prev_et[:, sh * 512:(sh + 1) * 512],
                           start=prev_ti == 0, stop=prev_ti == NT - 1)
                pend = [(ti, et)]
                e_sb.append(et)
            for prev_ti, prev_et in pend:
                for sh in range(NSH):
                    mm(rs_ps[sh], ones_k, prev_et[:, sh * 512:(sh + 1) * 512],
                       start=prev_ti == 0, stop=prev_ti == NT - 1)
            return e_sb, rs_ps

        def emit_av(n, vt_sb, e_sb, rs_ps):
            """Hn[c,s] = (sum_t Vt[t,c] E[t,s]) * (1/rowsum[s]).
            1/rowsum is broadcast across partitions by a DRAM-bounce DMA."""
            rinv = small.tile([1, S], F32, tag="rinv", name="rinv")
            for sh in range(NSH):
                nc.vector.reciprocal_approx_fast(
                    rinv[:, sh * 512:(sh + 1) * 512], rs_ps[sh])
            rv_d = dram.tile([1, S], F32, tag="rv_d", name="rv_d")
            nc.sync.dma_start(out=rv_d, in_=rinv)
            rbc_ps = [None, None]
            for sh in range(NSH):
                rbc_ps[sh] = rbc_pool.tile([128, 512], F32, tag="rbc",
                                           name="rbc")
                rh = rv_d[0:1, sh * 512:(sh + 1) * 512]
                bsrc = bass.AP(tensor=rh.tensor, offset=rh.offset,
                               ap=[[0, 128]] + [list(a) for a in rh.ap][1:])
                nc.sync.dma_start(out=rbc_ps[sh], in_=bsrc)
            hn_sb = []
            for ci in range(NCCH):
                hn = hn_pool.tile([128, S], BF16, tag="hn", name="hn")
                for sh in range(NSH):
                    ps = pmm.tile([128, 512], F32, tag="m", name="m")
                    for ti in range(NT):
                        mm(ps, vt_sb[ti][:, ci * 128:(ci + 1) * 128],
                           e_sb[ti][:, sh * 512:(sh + 1) * 512],
                           start=ti == 0, stop=ti == NT - 1)
                    nc.vector.tensor_tensor(hn[:, sh * 512:(sh + 1) * 512],
                                            ps, rbc_ps[sh], op=OP.mult)
                hn_sb.append(hn)
            return hn_sb

        def emit_o(n, hn_sb):
            for oi in range(NCCH):
                xp = xo_pool.tile([128, S], F32, tag="xp", name="xp")
                nc.sync.dma_start(out=xp,
                                  in_=xpb_ext[n, oi * 128:(oi + 1) * 128, :])
                ob = xo_pool.tile([128, S], F32, tag="o", name="o")
                for sh in range(NSH):
                    ps = pmm.tile([128, 512], F32, tag="m", name="m")
                    for ci in range(NCCH):
                        mm(ps, w_sb["woT"][:, ci, oi * 128:(oi + 1) * 128],
                           hn_sb[ci][:, sh * 512:(sh + 1) * 512],
                           start=ci == 0, stop=ci == NCCH - 1)
                    nc.vector.tensor_add(ob[:, sh * 512:(sh + 1) * 512], ps,
                                         xp[:, sh * 512:(sh + 1) * 512])
                    nc.sync.dma_start(
                        out=out_ext[n, oi * 128:(oi + 1) * 128,
                                    sh * 512:(sh + 1) * 512],
                        in_=ob[:, sh * 512:(sh + 1) * 512])

        # Software pipeline across the two samples: sample n+1's GroupNorm
        # statistics (DVE) are emitted under sample n's scores phase, and its
        # PE-side finish under sample n's attention phase, so the PE stream
        # never stalls at the sample boundary.
        stats = [None] * NSAMP
        hs = [None] * NSAMP
        stats[0] = gn_stats(0)
        hs[0] = gn_finish(0, stats[0])
        for n in range(NSAMP):
            q_sb, k_sb, vt_sb = emit_qkv(n, hs[n])
            if n + 1 < NSAMP:
                stats[n + 1] = gn_stats(n + 1)
            e_sb, rs_ps = emit_scores(n, q_sb, k_sb)
            hn_sb = emit_av(n, vt_sb, e_sb, rs_ps)
            if n + 1 < NSAMP:
                hs[n + 1] = gn_finish(n + 1, stats[n + 1])
            emit_o(n, hn_sb)

    nc.finalize()
    return nc


def _prep(inputs):
    f = lambda v: np.ascontiguousarray(np.asarray(v), dtype=np.float32)
    x = f(inputs["x"]).reshape(N, C, S)
    wq, wk, wv, wo = f(inputs["wq"]), f(inputs["wk"]), f(inputs["wv"]), f(inputs["wo"])
    bq, bk, bv, bo = f(inputs["bq"]), f(inputs["bk"]), f(inputs["bv"]), f(inputs["bo"])
    gamma, beta = f(inputs["gamma"]), f(inputs["beta"])

    obias = wo @ bv + bo
    xpb = x + obias[None, :, None]
    gm8 = np.zeros((128, 8), np.float32)
    gm8[np.arange(128), np.arange(128) // 16] = 1.0

    import ml_dtypes
    bf = lambda a: np.ascontiguousarray(a, dtype=ml_dtypes.bfloat16)
    rep = {
        "wqT": bf(wq.T), "wkT": bf(wk.T),
        "wvT": bf(wv.T), "woT": bf(wo.T),
        "bqt": np.ascontiguousarray(bq.reshape(NCCH, 128).T),
        "bkt": np.ascontiguousarray(bk.reshape(NCCH, 128).T),
        "gt": np.ascontiguousarray(gamma.reshape(NCCH, 128).T),
        "bt": np.ascontiguousarray(beta.reshape(NCCH, 128).T),
        "gm8": gm8, "gm8T": np.ascontiguousarray(gm8.T),
    }
    in_maps = []
    for i in range(NCORES):
        m = dict(rep)
        m["x"] = bf(x[i * NSAMP:(i + 1) * NSAMP])
        m["xpb"] = np.ascontiguousarray(xpb[i * NSAMP:(i + 1) * NSAMP])
        in_maps.append(m)
    return in_maps


def _run(inputs, trace=False):
    from concourse.bass_utils import run_bass_kernel_spmd
    if "nc" not in _CACHE:
        _CACHE["nc"] = _build()
    in_maps = _prep(inputs)
    res = run_bass_kernel_spmd(_CACHE["nc"], in_maps,
                               core_ids=list(range(NCORES)), trace=trace)
    out = np.concatenate([res.results[i]["out"] for i in range(NCORES)], axis=0)
    return out.reshape(N, C, H, W), res


def kernel(**inputs) -> np.ndarray:
    out, _ = _run(inputs, trace=False)
    return out



# revision 7
# speedup vs baseline: 1.5290x; 1.5290x over previous
"""AttentionBlock (GroupNorm + 1x1-conv QKV self-attention + residual) on 8 TRN2 cores.

Data-parallel over batch: 16 samples -> 2 per NeuronCore, no collectives.
Per-sample layout is [C, S] = [512, 1024] with channels on partitions (4 chunks
of 128). All five GEMM groups (QK conv, V conv, scores, AV, O-proj) run as
fp8e4 DoubleRow matmuls (2 contraction subtiles of 128 per instruction, 2x PE
ALU rate) with fp32 PSUM accumulation. GroupNorm statistics are folded on the
host into per-channel affine coefficients; the device only applies h = a*x+b.
The residual + output bias are folded on the host into xob = 64*(x + Wo@bv +
bo) in bf16, so the O-projection drain is a single DVE add and the host
un-scales the output by exactly 1/64. Weights are pre-scaled by 8 so fp8
operands sit in a good exponent range; the exp activation applies the
compensating 1/64 inside its scale and a -4*ln2 bias keeps E <= 25 (fp8e4 max
is 240). Softmax row-sums over the partition axis are a ones-vector DoubleRow
matmul; the 1/rowsum normalizer is broadcast across partitions with a K=1
fp32 matmul (outer product with a ones column) instead of a DRAM bounce.
"""

import numpy as np

N, C, H, W = 16, 512, 32, 32
S = H * W                      # 1024
NCORES = 8
NSAMP = N // NCORES            # 2 samples per core
NCCH = C // 128                # 4 channel chunks
NSH = S // 512                 # 2 free-dim halves
NT = S // 128                  # 8 key tiles
NPAIR = 2                      # contraction chunk pairs for DoubleRow (C)
GROUPS = 32
EPS = 1e-5
ALPHA = 8.0                    # host pre-scale on all four weight matrices
RESID = 64.0                   # host pre-scale on the residual (= ALPHA**2)
SCALE_EXP = float(C) ** -0.5 / (ALPHA * ALPHA)
EXP_BIAS = -2.772588722239781  # -4*ln2: keeps E = exp(z - 4ln2) <= ~25

_CACHE = {}


def _build():
    import concourse.bass as bass  # noqa: F401
    import concourse.tile as tile
    from concourse import bacc, mybir
    from contextlib import ExitStack

    F32 = mybir.dt.float32
    BF16 = mybir.dt.bfloat16
    F8 = mybir.dt.float8e4
    AF = mybir.ActivationFunctionType
    OP = mybir.AluOpType
    DR = mybir.MatmulPerfMode.DoubleRow

    nc = bacc.Bacc("TRN2", target_bir_lowering=False, debug=False,
                   num_devices=NCORES)

    xob_ext = nc.declare_dram_parameter("xob", [NSAMP, C, S], BF16, isOutput=False)
    wq8_ext = nc.declare_dram_parameter("wq8", [C, C], F8, isOutput=False)
    wk8_ext = nc.declare_dram_parameter("wk8", [C, C], F8, isOutput=False)
    wv8_ext = nc.declare_dram_parameter("wv8", [C, C], F8, isOutput=False)
    wo8_ext = nc.declare_dram_parameter("wo8", [C, C], F8, isOutput=False)
    bq8_ext = nc.declare_dram_parameter("bq8", [128, NCCH], F32, isOutput=False)
    bk8_ext = nc.declare_dram_parameter("bk8", [128, NCCH], F32, isOutput=False)
    ga_ext = nc.declare_dram_parameter("ga", [NSAMP, 128, NCCH], F32, isOutput=False)
    gb_ext = nc.declare_dram_parameter("gb", [NSAMP, 128, NCCH], F32, isOutput=False)
    out_ext = nc.declare_dram_parameter("out", [NSAMP, C, S], BF16, isOutput=True)

    with ExitStack() as ctx:
        tc = ctx.enter_context(tile.TileContext(nc))

        singles = ctx.enter_context(tc.tile_pool(name="singles", bufs=1))
        x_pool = ctx.enter_context(tc.tile_pool(name="x", bufs=2 * NCCH))
        h_pool = ctx.enter_context(tc.tile_pool(name="h", bufs=2))
        q_pool = ctx.enter_context(tc.tile_pool(name="q", bufs=2))
        k_pool = ctx.enter_context(tc.tile_pool(name="k", bufs=2))
        v_pool = ctx.enter_context(tc.tile_pool(name="v", bufs=2))
        e_pool = ctx.enter_context(tc.tile_pool(name="e", bufs=2))
        hn_pool = ctx.enter_context(tc.tile_pool(name="hn", bufs=2))
        o_pool = ctx.enter_context(tc.tile_pool(name="o", bufs=4))
        rbc_pool = ctx.enter_context(tc.tile_pool(name="rbc", bufs=2))
        small = ctx.enter_context(tc.tile_pool(name="small", bufs=4))
        pmm = ctx.enter_context(tc.tile_pool(name="pmm", bufs=4, space="PSUM"))
        prs = ctx.enter_context(tc.tile_pool(name="prs", bufs=2, space="PSUM"))
        prb = ctx.enter_context(tc.tile_pool(name="prb", bufs=2, space="PSUM"))

        # --- PE warmup: ~10 independent zero matmuls keep the PE busy during
        # --- the head DMAs so the HAM clock gate is at 8/8 when real MMs start
        wu = singles.tile([128, 512], BF16, tag="wu", name="wu")
        nc.vector.memset(wu, 0.0)
        for _ in range(10):
            wps = pmm.tile([128, 512], F32, tag="m", name="m")
            nc.tensor.matmul(wps, wu[:, 0:128], wu, start=True, stop=True)

        # --- DMA priority order: sample-0 x (gates the affine), small
        # --- constants, wq/wk (gate the first conv), sample-1 x, wv, wo.
        xcs = [[None] * NCCH for _ in range(NSAMP)]

        def fetch_x(n):
            for ci in range(NCCH):
                xc = x_pool.tile([128, S], BF16, tag="x", name="x")
                nc.sync.dma_start(out=xc,
                                  in_=xob_ext[n, ci * 128:(ci + 1) * 128, :])
                xcs[n][ci] = xc

        fetch_x(0)
        bq8_sb = singles.tile([128, NCCH], F32, tag="bq8", name="bq8")
        nc.sync.dma_start(out=bq8_sb, in_=bq8_ext[:])
        bk8_sb = singles.tile([128, NCCH], F32, tag="bk8", name="bk8")
        nc.sync.dma_start(out=bk8_sb, in_=bk8_ext[:])
        ga_sb, gb_sb = [], []
        for n in range(NSAMP):
            g = singles.tile([128, NCCH], F32, tag=f"ga{n}", name=f"ga{n}")
            nc.sync.dma_start(out=g, in_=ga_ext[n])
            ga_sb.append(g)
            g = singles.tile([128, NCCH], F32, tag=f"gb{n}", name=f"gb{n}")
            nc.sync.dma_start(out=g, in_=gb_ext[n])
            gb_sb.append(g)
        ones2 = singles.tile([128, NPAIR, 16], F8, tag="ones2", name="ones2")
        nc.vector.memset(ones2, 1.0)
        ebias = singles.tile([128, 1], F32, tag="ebias", name="ebias")
        nc.vector.memset(ebias, EXP_BIAS)
        onec = singles.tile([1, 128], F32, tag="onec", name="onec")
        nc.vector.memset(onec, 1.0)

        w_sb = {}

        def fetch_w(name, ext):
            t = singles.tile([128, NCCH, C], F8, tag=name, name=name)
            nc.sync.dma_start(out=t, in_=ext.ap().rearrange(
                "(a p) o -> p a o", p=128))
            w_sb[name] = t

        fetch_w("wq8", wq8_ext)
        fetch_w("wk8", wk8_ext)
        fetch_x(1)
        fetch_w("wv8", wv8_ext)
        fetch_w("wo8", wo8_ext)

        def mmdr(ps, lhsT, rhs, start, stop):
            nc.tensor.matmul(ps, lhsT, rhs, start=start, stop=stop,
                             perf_mode=DR)

        def affine(n):
            """h = ga*xob + gb per channel chunk, quantized to fp8."""
            h8 = h_pool.tile([128, NCCH, S], F8, tag="h", name="h")
            for ci in range(NCCH):
                nc.vector.tensor_scalar(out=h8[:, ci, :], in0=xcs[n][ci],
                                        scalar1=ga_sb[n][:, ci:ci + 1],
                                        scalar2=gb_sb[n][:, ci:ci + 1],
                                        op0=OP.mult, op1=OP.add)
            return h8

        def emit_qk(n, h8):
            q8 = q_pool.tile([128, NCCH, S], F8, tag="q", name="q")
            k8 = k_pool.tile([128, NCCH, S], F8, tag="k", name="k")
            for wname, bias_sb, dst in (("wq8", bq8_sb, q8),
                                        ("wk8", bk8_sb, k8)):
                w = w_sb[wname]
                for oi in range(NCCH):
                    for sh in range(NSH):
                        ps = pmm.tile([128, 512], F32, tag="m", name="m")
                        for j in range(NPAIR):
                            mmdr(ps, w[:, 2 * j:2 * j + 2,
                                       oi * 128:(oi + 1) * 128],
                                 h8[:, 2 * j:2 * j + 2,
                                    sh * 512:(sh + 1) * 512],
                                 start=j == 0, stop=j == NPAIR - 1)
                        nc.vector.tensor_scalar(
                            out=dst[:, oi, sh * 512:(sh + 1) * 512], in0=ps,
                            scalar1=bias_sb[:, oi:oi + 1], scalar2=None,
                            op0=OP.add)
            return q8, k8

        def emit_v(n, h8):
            v8 = v_pool.tile([128, NT, C], F8, tag="v", name="v")
            for ti in range(NT):
                ps = pmm.tile([128, 512], F32, tag="m", name="m")
                for j in range(NPAIR):
                    mmdr(ps, h8[:, 2 * j:2 * j + 2, ti * 128:(ti + 1) * 128],
                         w_sb["wv8"][:, 2 * j:2 * j + 2, :],
                         start=j == 0, stop=j == NPAIR - 1)
                nc.vector.tensor_copy(v8[:, ti, :], ps)
            return v8

        def emit_scores(n, q8, k8):
            """St[t,s] = K^T Q (x64), E = exp(St/(64*sqrt(C)) - 4ln2) in fp8;
            rowsum over t via a ones DoubleRow matmul, emitted two key-tiles
            behind the scores so the PE never waits on the Exp activation."""
            e8 = e_pool.tile([128, NT, S], F8, tag="e", name="e")
            rs = [prs.tile([1, 512], F32, tag="r", name="r")
                  for _ in range(NSH)]

            def rowsum(j):
                for sh in range(NSH):
                    mmdr(rs[sh], ones2[:, :, 0:1],
                         e8[:, 2 * j:2 * j + 2, sh * 512:(sh + 1) * 512],
                         start=j == 0, stop=j == NT // 2 - 1)

            for ti in range(NT):
                for sh in range(NSH):
                    ps = pmm.tile([128, 512], F32, tag="m", name="m")
                    for i in range(NPAIR):
                        mmdr(ps, k8[:, 2 * i:2 * i + 2,
                                    ti * 128:(ti + 1) * 128],
                             q8[:, 2 * i:2 * i + 2, sh * 512:(sh + 1) * 512],
                             start=i == 0, stop=i == NPAIR - 1)
                    nc.scalar.activation(e8[:, ti, sh * 512:(sh + 1) * 512],
                                         ps, AF.Exp, bias=ebias,
                                         scale=SCALE_EXP)
                if ti >= 3 and ti % 2 == 1:
                    rowsum((ti - 3) // 2)
            rowsum(NT // 2 - 1)
            return e8, rs

        def emit_bcast(rs):
            """1/rowsum, broadcast to all 128 partitions via a K=1 fp32
            matmul (ones column outer product), staged to SBUF so the AV
            drain reads only one PSUM operand."""
            rinv = small.tile([1, S], F32, tag="rinv", name="rinv")
            rbc = []
            for sh in range(NSH):
                nc.vector.reciprocal_approx_fast(
                    rinv[:, sh * 512:(sh + 1) * 512], rs[sh])
                rb = prb.tile([128, 512], F32, tag="rb", name="rb")
                nc.tensor.matmul(rb, onec, rinv[:, sh * 512:(sh + 1) * 512],
                                 start=True, stop=True)
                rb_sb = rbc_pool.tile([128, 512], F32, tag="rbs", name="rbs")
                nc.vector.tensor_copy(rb_sb, rb)
                rbc.append(rb_sb)
            return rbc

        def emit_av(n, v8, e8, rbc):
            hn8 = hn_pool.tile([128, NCCH, S], F8, tag="hn", name="hn")
            for ci in range(NCCH):
                for sh in range(NSH):
                    ps = pmm.tile([128, 512], F32, tag="m", name="m")
                    for j in range(NT // 2):
                        mmdr(ps, v8[:, 2 * j:2 * j + 2,
                                    ci * 128:(ci + 1) * 128],
                             e8[:, 2 * j:2 * j + 2, sh * 512:(sh + 1) * 512],
                             start=j == 0, stop=j == NT // 2 - 1)
                    nc.vector.tensor_tensor(
                        hn8[:, ci, sh * 512:(sh + 1) * 512], ps, rbc[sh],
                        op=OP.mult)
            return hn8

        def emit_o(n, hn8):
            for oi in range(NCCH):
                xf = o_pool.tile([128, S], F32, tag="xf", name="xf")
                nc.vector.tensor_copy(xf, xcs[n][oi])
                ob = o_pool.tile([128, S], BF16, tag="o", name="o")
                for sh in range(NSH):
                    ps = pmm.tile([128, 512], F32, tag="m", name="m")
                    for i in range(NPAIR):
                        mmdr(ps, w_sb["wo8"][:, 2 * i:2 * i + 2,
                                             oi * 128:(oi + 1) * 128],
                             hn8[:, 2 * i:2 * i + 2, sh * 512:(sh + 1) * 512],
                             start=i == 0, stop=i == NPAIR - 1)
                    nc.vector.tensor_tensor(
                        ob[:, sh * 512:(sh + 1) * 512], ps,
                        xf[:, sh * 512:(sh + 1) * 512], op=OP.add)
                nc.sync.dma_start(out=out_ext[n, oi * 128:(oi + 1) * 128, :],
                                  in_=ob)

        h8 = [None] * NSAMP
        h8[0] = affine(0)
        for n in range(NSAMP):
            q8, k8 = emit_qk(n, h8[n])
            v8 = emit_v(n, h8[n])
            if n + 1 < NSAMP:
                h8[n + 1] = affine(n + 1)
            e8, rs = emit_scores(n, q8, k8)
            rbc = emit_bcast(rs)
            hn8 = emit_av(n, v8, e8, rbc)
            emit_o(n, hn8)

    nc.finalize()
    return nc


def _prep(inputs):
    import ml_dtypes
    f = lambda v: np.ascontiguousarray(np.asarray(v), dtype=np.float32)
    x = f(inputs["x"]).reshape(N, C, S)
    wq, wk, wv, wo = f(inputs["wq"]), f(inputs["wk"]), f(inputs["wv"]), f(inputs["wo"])
    bq, bk, bv, bo = f(inputs["bq"]), f(inputs["bk"]), f(inputs["bv"]), f(inputs["bo"])
    gamma, beta = f(inputs["gamma"]), f(inputs["beta"])

    # GroupNorm statistics on host -> per-channel affine h = a*x + b
    xr = x.reshape(N, GROUPS, (C // GROUPS) * S)
    mean = xr.mean(axis=2)                       # [N, 32]
    var = xr.var(axis=2)
    rstd = 1.0 / np.sqrt(var + EPS)
    a_pc = gamma[None, :] * np.repeat(rstd, C // GROUPS, axis=1)   # [N, C]
    b_pc = beta[None, :] - np.repeat(mean, C // GROUPS, axis=1) * a_pc

    # Residual fold: xob = 64*(x + obias); affine compensated so that
    # ga*xob + gb == a*x + b exactly.
    obias = wo @ bv + bo                         # [C]
    xob = (x + obias[None, :, None]) * RESID
    ga = a_pc / RESID                            # [N, C]
    gb = b_pc - a_pc * obias[None, :]

    bf = lambda a: np.ascontiguousarray(a, dtype=ml_dtypes.bfloat16)
    f8 = lambda a: np.ascontiguousarray(a, dtype=ml_dtypes.float8_e4m3)
    col = lambda a: np.ascontiguousarray(a.reshape(NCCH, 128).T)
    rep = {
        "wq8": f8(ALPHA * wq.T), "wk8": f8(ALPHA * wk.T),
        "wv8": f8(ALPHA * wv.T), "wo8": f8(ALPHA * wo.T),
        "bq8": col(ALPHA * bq), "bk8": col(ALPHA * bk),
    }
    in_maps = []
    for i in range(NCORES):
        m = dict(rep)
        sl = slice(i * NSAMP, (i + 1) * NSAMP)
        m["xob"] = bf(xob[sl])
        m["ga"] = np.ascontiguousarray(
            np.stack([col(ga[j]) for j in range(i * NSAMP, (i + 1) * NSAMP)]))
        m["gb"] = np.ascontiguousarray(
            np.stack([col(gb[j]) for j in range(i * NSAMP, (i + 1) * NSAMP)]))
        in_maps.append(m)
    return in_maps


def _run(inputs, trace=False):
    from concourse.bass_utils import run_bass_kernel_spmd
    if "nc" not in _CACHE:
        _CACHE["nc"] = _build()
    in_maps = _prep(inputs)
    res = run_bass_kernel_spmd(_CACHE["nc"], in_maps,
                               core_ids=list(range(NCORES)), trace=trace)
    out = np.concatenate([np.asarray(res.results[i]["out"], dtype=np.float32)
                          for i in range(NCORES)], axis=0)
    out *= 1.0 / RESID
    return out.reshape(N, C, H, W), res


def kernel(**inputs) -> np.ndarray:
    out, _ = _run(inputs, trace=False)
    return out
